# revision 18
# baseline (speedup 1.0000x reference)
"""Trainium2 Bass kernel for nn_Linear_80874234183916.

Computes y = x @ w_eff.T + bias where w_eff keeps only the weight entries
whose |w| is >= the median of |w| (top max_iter = n/2 entries by magnitude;
budgeted approximate matmul).

Sharding: tensor-parallel over out_features across 8 NeuronCores — each core
owns a 512-column slice of the output, masks its own weight slice on device,
and computes x @ w_slice_eff.T + bias_slice. x is replicated (streamed).

Host-side work is limited to:
  - the order statistic (threshold = k-th largest |w|) via np.partition —
    a selection over 16.7M elements that has no efficient mapping onto the
    TRN2 engines; the threshold is baked into the NEFF as an immediate.
  - layout prep (transpose/tiling of x and w so every device DMA is a
    contiguous, full-partition transfer) and the final concat of the 8
    output slices.

All O(N*K*M) compute (matmul), the O(K*M) masking, and the bias add run on
device. The matmul uses float32r (full fp32 operands, 1 row/cycle on the PE
at moving-dim >= 256; ~1e-4 relative error vs fp64 at K=4096).
"""

import numpy as np

import concourse.bass as bass
import concourse.mybir as mybir
import concourse.tile as tile
from concourse import bacc
from concourse.bass_utils import run_bass_kernel_spmd

N_TOK = 8192
IN_F = 4096
OUT_F = 4096
N_CORES = 8
O_S = OUT_F // N_CORES  # 512 out-features per core
P = 128
KO = IN_F // P          # 32 k-chunks
TT = N_TOK // P         # 64 token tiles
X_BUFS = 6
MAX_ITER = IN_F * OUT_F // 2

dt = mybir.dt


def _build(thresh: float, reps: int = 1, ntt: int = TT, nox: bool = False,
           mm_dtype: "mybir.dt" = None, noevict: bool = False):
    """Build the per-core Bass program (SPMD: same NEFF, per-core data).

    reps>1 repeats the token-tile loop and ntt limits the number of token
    tiles; nox reuses a single x tile for every token tile (wrong results —
    timing experiments only); mm_dtype overrides the matmul input dtype for
    timing experiments (operands are memset, results wrong).
    """
    nc = bacc.Bacc("TRN2", target_bir_lowering=False, debug=False)

    # Host pre-tiled layouts (see kernel() for the exact host-side packing):
    #   xt[tt, ki, ko, t] = x[tt*128 + t, ko*128 + ki]
    #   wt[ki, ko, n]     = w_slice[n, ko*128 + ki]
    # x never touches an ALU, so it is declared float32r end to end; w is
    # loaded as float32, masked, and the final multiply rounds into float32r.
    f32r = dt.float32r
    xt = nc.dram_tensor("xt", [TT, P, KO, P], f32r, kind="ExternalInput").ap()
    wt = nc.dram_tensor("wt", [P, KO, O_S], dt.float32, kind="ExternalInput").ap()
    bb = nc.dram_tensor("bb", [P, O_S], dt.float32, kind="ExternalInput").ap()
    y = nc.dram_tensor("y", [N_TOK, O_S], dt.float32, kind="ExternalOutput").ap()

    with tile.TileContext(nc) as tc:
        with (
            tc.tile_pool(name="wpool", bufs=1) as wpool,
            tc.tile_pool(name="wcpool", bufs=3) as wcpool,
            tc.tile_pool(name="xpool", bufs=X_BUFS) as xpool,
            tc.tile_pool(name="mpool", bufs=4) as mpool,
            tc.tile_pool(name="opool", bufs=4) as opool,
            tc.tile_pool(name="cpool", bufs=1) as cpool,
            tc.tile_pool(name="pspool", bufs=4, space="PSUM") as ps,
        ):
            bias_sb = cpool.tile([P, O_S], dt.float32, tag="bias")
            nc.sync.dma_start(bias_sb[:], bb)

            # Load weight slice chunk-wise, mask (w_eff = w * (|w| >= t)),
            # and round into the resident float32r tile the matmuls consume.
            wm_sb = wpool.tile([P, KO, O_S], f32r, tag="wm")
            for ko in range(KO):
                wc = wcpool.tile([P, O_S], dt.float32, tag="wc")
                nc.sync.dma_start(wc[:], wt[:, ko])
                m_sb = mpool.tile([P, O_S], dt.float32, tag="mask")
                nc.scalar.activation(
                    m_sb[:], wc[:], mybir.ActivationFunctionType.Abs
                )
                nc.vector.tensor_scalar(
                    m_sb[:], m_sb[:], float(thresh), None, mybir.AluOpType.is_ge
                )
                nc.vector.tensor_mul(wm_sb[:, ko], wc[:], m_sb[:])

            if mm_dtype is not None:
                # dtype timing experiment: memset operands, same MM stream
                wm_sb = wpool.tile([P, KO, O_S], mm_dtype, tag="wme")
                nc.vector.memset(wm_sb[:], 0.5)

            x_fixed = None
            if nox:
                if mm_dtype is not None:
                    x_fixed = xpool.tile([P, KO, P], mm_dtype, tag="x")
                    nc.vector.memset(x_fixed[:], 0.5)
                else:
                    x_fixed = xpool.tile([P, KO, P], f32r, tag="x")
                    nc.sync.dma_start(x_fixed[:], xt[0])
            for _rep in range(reps):
              for tt in range(ntt):
                if nox:
                    x_sb = x_fixed
                else:
                    x_sb = xpool.tile([P, KO, P], f32r, tag="x")
                    nc.sync.dma_start(x_sb[:], xt[tt])
                psum = ps.tile([P, O_S], dt.float32, tag="ps")
                for ko in range(KO):
                    nc.tensor.matmul(
                        psum[:],
                        x_sb[:, ko],
                        wm_sb[:, ko],
                        start=(ko == 0),
                        stop=(ko == KO - 1),
                    )
                if not noevict:
                    out_sb = opool.tile([P, O_S], dt.float32, tag="out")
                    nc.vector.tensor_add(out_sb[:], psum[:], bias_sb[:])
                    nc.sync.dma_start(y[tt * P : (tt + 1) * P, :], out_sb[:])

    nc.compile()
    return nc


def _prep_inputs(x, weight, bias):
    """Host-side: threshold + per-core DMA-friendly layouts."""
    flat_abs = np.abs(weight.reshape(-1))
    k = flat_abs.size - MAX_ITER
    thresh = float(np.partition(flat_abs, k)[k])

    # xt[tt, ki, ko, t] = x[tt*128+t, ko*128+ki]
    xt = np.ascontiguousarray(
        x.reshape(TT, P, KO, P).transpose(0, 3, 2, 1)
    )

    in_maps = []
    for c in range(N_CORES):
        w_s = weight[c * O_S : (c + 1) * O_S]  # [O_S, IN_F]
        # wt[ki, ko, n] = w_s[n, ko*128+ki]
        wt = np.ascontiguousarray(w_s.reshape(O_S, KO, P).transpose(2, 1, 0))
        bb = np.ascontiguousarray(
            np.broadcast_to(bias[c * O_S : (c + 1) * O_S], (P, O_S))
        )
        in_maps.append({"xt": xt, "wt": wt, "bb": bb})
    return thresh, in_maps


def _run(x, weight, bias, trace=False, **run_kwargs):
    x = np.asarray(x, dtype=np.float32)
    weight = np.asarray(weight, dtype=np.float32)
    bias = np.asarray(bias, dtype=np.float32)
    assert x.shape == (N_TOK, IN_F) and weight.shape == (OUT_F, IN_F)

    thresh, in_maps = _prep_inputs(x, weight, bias)
    nc = _build(thresh)
    res = run_bass_kernel_spmd(
        nc, in_maps, core_ids=list(range(N_CORES)), trace=trace, **run_kwargs
    )
    y = np.concatenate([r["y"] for r in res.results], axis=1)
    return y, res


def kernel(x, weight, bias):
    y, _ = _run(x, weight, bias, trace=False)
    return y


# revision 19
# speedup vs baseline: 1.6383x; 1.6383x over previous
"""Trainium2 Bass kernel for nn_Linear_80874234183916.

Computes y = x @ w_eff.T + bias where w_eff keeps only the weight entries
whose |w| is >= the median of |w| (top max_iter = n/2 entries by magnitude;
budgeted approximate matmul).

Sharding: tensor-parallel over out_features across 8 NeuronCores — each core
owns a 512-column slice of the output, masks its own weight slice on device,
and computes x @ w_slice_eff.T + bias_slice. x is replicated (streamed).

Host-side work is limited to:
  - the order statistic (threshold = k-th largest |w|) via np.partition —
    a selection over 16.7M elements that has no efficient mapping onto the
    TRN2 engines; the threshold is baked into the NEFF as an immediate.
  - layout prep (transpose/tiling of x and w so every device DMA is a
    contiguous, full-partition transfer) and the final concat of the 8
    output slices.

All O(N*K*M) compute (matmul), the O(K*M) masking, and the bias add run on
device. The matmul uses float32r (full fp32 operands, 1 row/cycle on the PE
at moving-dim >= 256; ~1e-4 relative error vs fp64 at K=4096).
"""

import numpy as np

import concourse.bass as bass
import concourse.mybir as mybir
import concourse.tile as tile
from concourse import bacc
from concourse.bass_utils import run_bass_kernel_spmd

N_TOK = 8192
IN_F = 4096
OUT_F = 4096
N_CORES = 8
O_S = OUT_F // N_CORES  # 512 out-features per core
P = 128
KO = IN_F // P          # 32 k-chunks
TT = N_TOK // P         # 64 token tiles
X_BUFS = 6
MAX_ITER = IN_F * OUT_F // 2

dt = mybir.dt


def _build(thresh: float, reps: int = 1, ntt: int = TT, nox: bool = False,
           mm_dtype: "mybir.dt" = None, noevict: bool = False,
           ps_bufs: int = 4):
    """Build the per-core Bass program (SPMD: same NEFF, per-core data).

    reps>1 repeats the token-tile loop and ntt limits the number of token
    tiles; nox reuses a single x tile for every token tile (wrong results —
    timing experiments only); mm_dtype overrides the matmul input dtype for
    timing experiments (operands are memset, results wrong).
    """
    nc = bacc.Bacc("TRN2", target_bir_lowering=False, debug=False)

    # Host pre-tiled layouts (see kernel() for the exact host-side packing):
    #   xt[tt, ki, ko, t] = x[tt*128 + t, ko*128 + ki]
    #   wt[ki, ko, n]     = w_slice[n, ko*128 + ki]
    # x never touches an ALU, so it is declared float32r end to end; w is
    # loaded as float32, masked, and the final multiply rounds into float32r.
    f32r = dt.float32r
    xt = nc.dram_tensor("xt", [TT, P, KO, P], f32r, kind="ExternalInput").ap()
    wt = nc.dram_tensor("wt", [P, KO, O_S], dt.float32, kind="ExternalInput").ap()
    bb = nc.dram_tensor("bb", [P, O_S], dt.float32, kind="ExternalInput").ap()
    y = nc.dram_tensor("y", [N_TOK, O_S], dt.float32, kind="ExternalOutput").ap()

    with tile.TileContext(nc) as tc:
        with (
            tc.tile_pool(name="wpool", bufs=1) as wpool,
            tc.tile_pool(name="wcpool", bufs=3) as wcpool,
            tc.tile_pool(name="xpool", bufs=X_BUFS) as xpool,
            tc.tile_pool(name="mpool", bufs=4) as mpool,
            tc.tile_pool(name="opool", bufs=4) as opool,
            tc.tile_pool(name="cpool", bufs=1) as cpool,
            tc.tile_pool(name="pspool", bufs=ps_bufs, space="PSUM") as ps,
        ):
            bias_sb = cpool.tile([P, O_S], dt.float32, tag="bias")
            nc.sync.dma_start(bias_sb[:], bb)

            # Load weight slice chunk-wise, mask (w_eff = w * (|w| >= t)),
            # and round into the resident float32r tile the matmuls consume.
            wm_sb = wpool.tile([P, KO, O_S], f32r, tag="wm")
            for ko in range(KO):
                wc = wcpool.tile([P, O_S], dt.float32, tag="wc")
                nc.sync.dma_start(wc[:], wt[:, ko])
                m_sb = mpool.tile([P, O_S], dt.float32, tag="mask")
                nc.scalar.activation(
                    m_sb[:], wc[:], mybir.ActivationFunctionType.Abs
                )
                nc.vector.tensor_scalar(
                    m_sb[:], m_sb[:], float(thresh), None, mybir.AluOpType.is_ge
                )
                nc.vector.tensor_mul(wm_sb[:, ko], wc[:], m_sb[:])

            if mm_dtype is not None:
                # dtype timing experiment: memset operands, same MM stream
                wm_sb = wpool.tile([P, KO, O_S], mm_dtype, tag="wme")
                nc.vector.memset(wm_sb[:], 0.5)

            x_fixed = None
            if nox:
                if mm_dtype is not None:
                    x_fixed = xpool.tile([P, KO, P], mm_dtype, tag="x")
                    nc.vector.memset(x_fixed[:], 0.5)
                else:
                    x_fixed = xpool.tile([P, KO, P], f32r, tag="x")
                    nc.sync.dma_start(x_fixed[:], xt[0])
            for _rep in range(reps):
              for tt in range(ntt):
                if nox:
                    x_sb = x_fixed
                else:
                    x_sb = xpool.tile([P, KO, P], f32r, tag="x")
                    nc.sync.dma_start(x_sb[:], xt[tt])
                psum = ps.tile([P, O_S], dt.float32, tag="ps")
                for ko in range(KO):
                    nc.tensor.matmul(
                        psum[:],
                        x_sb[:, ko],
                        wm_sb[:, ko],
                        start=(ko == 0),
                        stop=(ko == KO - 1),
                    )
                if not noevict:
                    out_sb = opool.tile([P, O_S], dt.float32, tag="out")
                    nc.vector.tensor_add(out_sb[:], psum[:], bias_sb[:])
                    nc.sync.dma_start(y[tt * P : (tt + 1) * P, :], out_sb[:])

    nc.compile()
    return nc


def _prep_inputs(x, weight, bias):
    """Host-side: threshold + per-core DMA-friendly layouts."""
    flat_abs = np.abs(weight.reshape(-1))
    k = flat_abs.size - MAX_ITER
    thresh = float(np.partition(flat_abs, k)[k])

    # xt[tt, ki, ko, t] = x[tt*128+t, ko*128+ki]
    xt = np.ascontiguousarray(
        x.reshape(TT, P, KO, P).transpose(0, 3, 2, 1)
    )

    in_maps = []
    for c in range(N_CORES):
        w_s = weight[c * O_S : (c + 1) * O_S]  # [O_S, IN_F]
        # wt[ki, ko, n] = w_s[n, ko*128+ki]
        wt = np.ascontiguousarray(w_s.reshape(O_S, KO, P).transpose(2, 1, 0))
        bb = np.ascontiguousarray(
            np.broadcast_to(bias[c * O_S : (c + 1) * O_S], (P, O_S))
        )
        in_maps.append({"xt": xt, "wt": wt, "bb": bb})
    return thresh, in_maps


def _run(x, weight, bias, trace=False, **run_kwargs):
    x = np.asarray(x, dtype=np.float32)
    weight = np.asarray(weight, dtype=np.float32)
    bias = np.asarray(bias, dtype=np.float32)
    assert x.shape == (N_TOK, IN_F) and weight.shape == (OUT_F, IN_F)

    thresh, in_maps = _prep_inputs(x, weight, bias)
    nc = _build(thresh)
    res = run_bass_kernel_spmd(
        nc, in_maps, core_ids=list(range(N_CORES)), trace=trace, **run_kwargs
    )
    y = np.concatenate([r["y"] for r in res.results], axis=1)
    return y, res


def kernel(x, weight, bias):
    y, _ = _run(x, weight, bias, trace=False)
    return y
